# revision 2
# baseline (speedup 1.0000x reference)
"""Lookahead depthwise convolution on 8 Trainium2 NeuronCores.

out[t, b, f] = sum_{c=0..K-1} x[t+c, b, f] * weight[f, c], zero-padded at the
right edge. x: (2048, 32, 1280) fp32, weight: (1280, 81) fp32.

Feature-sharded across 8 cores (160 features each). Per feature the time conv
over 128-wide time tiles is a banded Toeplitz matmul:
  out_i = A_f @ x_i + B_f @ x_{i+1}
with stationary lhsT_A[t_in, t_out] = w[f, t_in - t_out] (0 <= d < K) and
lhsT_B[t_in, t_out] = w[f, t_in + 128 - t_out].

The host pre-permutes x to fp16 [t(128), f, i, b] so all 16 time blocks of a
feature are 512 contiguous SBUF columns. Each feature then needs only two
matmuls (free dim 512 / 480) per stationary matrix — LDWEIGHTS is amortized
over every time block instead of being reloaded per block, and I/O is fp16
(half the HBM traffic). The B_f moving operand is the same 512 columns
offset by one block (32 cols); block 16 is past the sequence end (the conv is
zero-padded there), so B_f only contributes to out blocks 0..14 (480 cols).
fp16 x fp16 products accumulate exactly in the fp32 PSUM, so the only error
is fp16 rounding of inputs/outputs (~5e-4 rel).
"""

import numpy as np

import concourse.bass as bass
import concourse.bacc as bacc
import concourse.mybir as mybir
from concourse import tile
from concourse.bass_utils import run_bass_kernel_spmd

S, B, F, K = 2048, 32, 1280, 81
N_CORES = 8
FC = F // N_CORES        # features per core (160)
TB = S // 128            # time blocks (16)
CPF = TB * B             # moving columns per feature (512)
CF = 16                  # features per chunk
NCH = FC // CF           # chunks per core (10)

_compiled = None


def _build_program():
    nc = bacc.Bacc("TRN2", target_bir_lowering=False, debug=False)
    f32, f16 = mybir.dt.float32, mybir.dt.float16

    x_in = nc.declare_dram_parameter("x16", [128, FC * CPF], f16,
                                     isOutput=False)
    bands_in = nc.declare_dram_parameter("bands", [128, FC * 256], f16,
                                         isOutput=False)
    out_ext = nc.declare_dram_parameter("out16", [128, FC * CPF], f16,
                                        isOutput=True)

    with tile.TileContext(nc) as tc:
        with (
            tc.tile_pool(name="x", bufs=2) as xpool,
            tc.tile_pool(name="bands", bufs=2) as bpool,
            tc.tile_pool(name="out", bufs=2) as opool,
            tc.tile_pool(name="psum", bufs=8, space="PSUM") as ppool,
        ):
            for ch in range(NCH):
                xt = xpool.tile([128, CF * CPF], f16)
                nc.sync.dma_start(
                    out=xt[:],
                    in_=x_in[:, ch * CF * CPF:(ch + 1) * CF * CPF])
                bt = bpool.tile([128, CF * 256], f16)
                nc.sync.dma_start(
                    out=bt[:],
                    in_=bands_in[:, ch * CF * 256:(ch + 1) * CF * 256])
                ot = opool.tile([128, CF * CPF], f16)
                for j in range(CF):
                    psum = ppool.tile([128, CPF], f32)
                    lA = bt[:, j * 256:j * 256 + 128]
                    lB = bt[:, j * 256 + 128:j * 256 + 256]
                    nc.tensor.matmul(
                        out=psum[:], lhsT=lA,
                        rhs=xt[:, j * CPF:(j + 1) * CPF],
                        start=True, stop=False)
                    nc.tensor.matmul(
                        out=psum[:, 0:CPF - B], lhsT=lB,
                        rhs=xt[:, j * CPF + B:(j + 1) * CPF],
                        start=False, stop=True)
                    if j % 2 == 0:
                        nc.vector.tensor_copy(
                            out=ot[:, j * CPF:(j + 1) * CPF], in_=psum[:])
                    else:
                        nc.scalar.copy(
                            out=ot[:, j * CPF:(j + 1) * CPF], in_=psum[:])
                nc.sync.dma_start(
                    out=out_ext[:, ch * CF * CPF:(ch + 1) * CF * CPF],
                    in_=ot[:])
    nc.finalize()
    return nc


def _build_bands(weight):
    """Per-feature stationary matrices, laid out [t_in(128), (f, {A,B}, t_out)].

    lhsT_A[t_in, t_out] = w[f, t_in - t_out]        (0 <= d < K)
    lhsT_B[t_in, t_out] = w[f, t_in + 128 - t_out]  (0 <= d < K)
    """
    p = np.arange(128)[:, None]   # t_in
    m = np.arange(128)[None, :]   # t_out
    dA = p - m
    dB = p + 128 - m
    mA = (dA >= 0) & (dA < K)
    mB = (dB >= 0) & (dB < K)
    iA = np.clip(dA, 0, K - 1)
    iB = np.clip(dB, 0, K - 1)
    w16 = weight.astype(np.float16).astype(np.float32)
    A = w16[:, iA] * mA           # [F, t_in, t_out]
    Bm = w16[:, iB] * mB
    bands = np.empty((128, F, 2, 128), np.float16)
    bands[:, :, 0, :] = A.transpose(1, 0, 2)
    bands[:, :, 1, :] = Bm.transpose(1, 0, 2)
    return bands


def make_in_maps(x, weight):
    """Host-side shard + permute: per core fp16 x as [t, f, i, b] and bands."""
    x16 = np.asarray(x, dtype=np.float32).astype(np.float16)
    bands = _build_bands(np.asarray(weight, dtype=np.float32))
    in_maps = []
    for c in range(N_CORES):
        fl = slice(c * FC, (c + 1) * FC)
        xc = x16[:, :, fl].reshape(TB, 128, B, FC)      # (i, t, b, f)
        xc = np.ascontiguousarray(xc.transpose(1, 3, 0, 2))  # (t, f, i, b)
        in_maps.append({
            "x16": xc.reshape(128, FC * CPF),
            "bands": np.ascontiguousarray(
                bands[:, fl, :, :]).reshape(128, FC * 256),
        })
    return in_maps


def unshard_output(res):
    outs = []
    for c in range(N_CORES):
        oc = np.asarray(res.results[c]["out16"]).reshape(128, FC, TB, B)
        outs.append(oc.transpose(2, 0, 3, 1).reshape(S, B, FC))  # (s, b, f)
    return np.concatenate(outs, axis=2).astype(np.float32)


def kernel(x, weight):
    global _compiled
    if _compiled is None:
        _compiled = _build_program()
    in_maps = make_in_maps(x, weight)
    res = run_bass_kernel_spmd(_compiled, in_maps, list(range(N_CORES)))
    return unshard_output(res)


# revision 4
# speedup vs baseline: 1.1486x; 1.1486x over previous
"""Lookahead depthwise convolution on 8 Trainium2 NeuronCores.

out[t, b, f] = sum_{c=0..K-1} x[t+c, b, f] * weight[f, c], zero-padded at the
right edge. x: (2048, 32, 1280) fp32, weight: (1280, 81) fp32.

Feature-sharded across 8 cores (160 features each). Per feature the time conv
over 128-wide time tiles is a banded Toeplitz matmul:
  out_i = A_f @ x_i + B_f @ x_{i+1}
with stationary lhsT_A[t_in, t_out] = w[f, t_in - t_out] (0 <= d < K) and
lhsT_B[t_in, t_out] = w[f, t_in + 128 - t_out].

The host pre-permutes x to fp16 [t(128), f, i, b] so all 16 time blocks of a
feature are 512 contiguous SBUF columns. Each feature then needs only two
matmuls (free dim 512 / 480) per stationary matrix — LDWEIGHTS is amortized
over every time block instead of being reloaded per block, and I/O is fp16
(half the HBM traffic). The B_f moving operand is the same 512 columns
offset by one block (32 cols); block 16 is past the sequence end (the conv is
zero-padded there), so B_f only contributes to out blocks 0..14 (480 cols).
fp16 x fp16 products accumulate exactly in the fp32 PSUM, so the only error
is fp16 rounding of inputs/outputs (~5e-4 rel).
"""

import numpy as np

import concourse.bass as bass
import concourse.bacc as bacc
import concourse.mybir as mybir
from concourse import tile
from concourse.bass_utils import run_bass_kernel_spmd

S, B, F, K = 2048, 32, 1280, 81
N_CORES = 8
FC = F // N_CORES        # features per core (160)
TB = S // 128            # time blocks (16)
CPF = TB * B             # moving columns per feature (512)
CF = 16                  # features per chunk
NCH = FC // CF           # chunks per core (10)

_compiled = None


CPC = CF * (CPF + 256)       # fused x+bands columns per chunk


def _build_program():
    nc = bacc.Bacc("TRN2", target_bir_lowering=False, debug=False)
    f32, f16 = mybir.dt.float32, mybir.dt.float16

    # Fused input: per chunk, CF*512 x columns then CF*256 band columns, so
    # each chunk is one large contiguous DMA.
    xb_in = nc.declare_dram_parameter("xb", [128, NCH * CPC], f16,
                                      isOutput=False)
    out_ext = nc.declare_dram_parameter("out16", [128, FC * CPF], f16,
                                        isOutput=True)

    with tile.TileContext(nc) as tc:
        with (
            tc.tile_pool(name="xb", bufs=3) as xpool,
            tc.tile_pool(name="out", bufs=3) as opool,
            tc.tile_pool(name="psum", bufs=8, space="PSUM") as ppool,
        ):
            for ch in range(NCH):
                xbt = xpool.tile([128, CPC], f16)
                nc.sync.dma_start(
                    out=xbt[:], in_=xb_in[:, ch * CPC:(ch + 1) * CPC])
                xt = xbt[:, 0:CF * CPF]
                bt = xbt[:, CF * CPF:CPC]
                ot = opool.tile([128, CF * CPF], f16)
                for j in range(CF):
                    psum = ppool.tile([128, CPF], f32)
                    lA = bt[:, j * 256:j * 256 + 128]
                    lB = bt[:, j * 256 + 128:j * 256 + 256]
                    nc.tensor.matmul(
                        out=psum[:], lhsT=lA,
                        rhs=xt[:, j * CPF:(j + 1) * CPF],
                        start=True, stop=False)
                    nc.tensor.matmul(
                        out=psum[:, 0:CPF - B], lhsT=lB,
                        rhs=xt[:, j * CPF + B:(j + 1) * CPF],
                        start=False, stop=True)
                    if j % 2 == 0:
                        nc.vector.tensor_copy(
                            out=ot[:, j * CPF:(j + 1) * CPF], in_=psum[:])
                    else:
                        nc.scalar.copy(
                            out=ot[:, j * CPF:(j + 1) * CPF], in_=psum[:])
                nc.sync.dma_start(
                    out=out_ext[:, ch * CF * CPF:(ch + 1) * CF * CPF],
                    in_=ot[:])
    nc.finalize()
    return nc


def _build_bands(weight):
    """Per-feature stationary matrices, laid out [t_in(128), (f, {A,B}, t_out)].

    lhsT_A[t_in, t_out] = w[f, t_in - t_out]        (0 <= d < K)
    lhsT_B[t_in, t_out] = w[f, t_in + 128 - t_out]  (0 <= d < K)
    """
    p = np.arange(128)[:, None]   # t_in
    m = np.arange(128)[None, :]   # t_out
    dA = p - m
    dB = p + 128 - m
    mA = (dA >= 0) & (dA < K)
    mB = (dB >= 0) & (dB < K)
    iA = np.clip(dA, 0, K - 1)
    iB = np.clip(dB, 0, K - 1)
    w16 = weight.astype(np.float16).astype(np.float32)
    A = w16[:, iA] * mA           # [F, t_in, t_out]
    Bm = w16[:, iB] * mB
    bands = np.empty((128, F, 2, 128), np.float16)
    bands[:, :, 0, :] = A.transpose(1, 0, 2)
    bands[:, :, 1, :] = Bm.transpose(1, 0, 2)
    return bands


def make_in_maps(x, weight):
    """Host-side shard + permute: per core fp16 x as [t, f, i, b] fused with
    the band matrices, chunk-major so each chunk is one contiguous DMA."""
    x16 = np.asarray(x, dtype=np.float32).astype(np.float16)
    bands = _build_bands(np.asarray(weight, dtype=np.float32))
    in_maps = []
    for c in range(N_CORES):
        fl = slice(c * FC, (c + 1) * FC)
        xc = x16[:, :, fl].reshape(TB, 128, B, FC)      # (i, t, b, f)
        xc = xc.transpose(1, 3, 0, 2).reshape(128, NCH, CF * CPF)
        bc = bands[:, fl, :, :].reshape(128, NCH, CF * 256)
        xb = np.concatenate([xc, bc], axis=2)           # [128, NCH, CPC]
        in_maps.append({"xb": np.ascontiguousarray(xb).reshape(128, NCH * CPC)})
    return in_maps


def unshard_output(res):
    outs = []
    for c in range(N_CORES):
        oc = np.asarray(res.results[c]["out16"]).reshape(128, FC, TB, B)
        outs.append(oc.transpose(2, 0, 3, 1).reshape(S, B, FC))  # (s, b, f)
    return np.concatenate(outs, axis=2).astype(np.float32)


def kernel(x, weight):
    global _compiled
    if _compiled is None:
        _compiled = _build_program()
    in_maps = make_in_maps(x, weight)
    res = run_bass_kernel_spmd(_compiled, in_maps, list(range(N_CORES)))
    return unshard_output(res)
